# revision 5
# baseline (speedup 1.0000x reference)
"""Trainium2 Bass kernel for nn_LookupTableMy (embedding gathers + LSTM + window dots).

Computation (per sample b):
  e1 = emb[input1[b]]                 # [19, 128]
  h  = LSTM(e1)  (H=384, 19 steps)    # final hidden [384]
  e2 = emb[input2[b]]                 # [20, 128]
  s_j[k] = h[128j:128j+128] . e2[k]   # j=0..2, k=0..19
  rs[n]  = s_0[n] + s_1[n+1] + s_2[n+2]   n=0..17
  ms     = max_n rs[n]
  out    = log_softmax(ms * lin_w[:,0] + lin_b)   # [2]

Sharding: data-parallel over batch: 4096 samples -> 8 cores x 512.
Weights replicated; the embedding table is compacted per core on the host
(each core touches <= 19,968 of the 100k rows; unique rows are packed into
a [20480, 128] f16 table and indices remapped to int16).

v3 design (vs the 426us f32r baseline):
  - dma_gather (gpsimd SWDGE, mlp library) instead of 156 single-row
    indirect DMAs: e1 is 19 transpose-mode gathers (one per timestep,
    512 rows each) that write xT[d, t, b] DIRECTLY -- no PE transposes,
    no DVE copies. e2 is 4 flat gathers (2560 rows each). Transpose-mode
    gathers race across SWDGE queues (shared staging), so e1 stays on
    queue 0; flat e2 gathers spread over queues 1-3.
  - All matmul operands f16 (weights, x, h): rhs streams 2 elem/cycle,
    ~131 ns/MM at N=512 vs ~233 ns for f32r.
  - Gate-major z chunks [128 gate-dims, 512 batch] accumulate in PSUM in
    4 per-gate groups of 3 banks. Bias is pre-seeded into PSUM by a K=1
    matmul (biasT x ones-row), letting one ACTIVATE cover all 3 segments
    of a gate ([128, 1536]): ACT drops from 15x720ns to ~5x1573ns/step.
    o-gate group first so its ACT overlaps the c-path chain.
  - tanh(c) merged over segments; c/h single [128, 3, 512] f16 tiles
    (h slices feed next step's matmuls directly).
"""

import sys
from contextlib import ExitStack

for _p in ("/opt/trn_rl_repo",):
    if _p not in sys.path:
        sys.path.insert(0, _p)

import numpy as np

import concourse.bass as bass
import concourse.tile as tile
import concourse.bacc as bacc
import concourse.mybir as mybir
from concourse import bass_utils
from concourse.masks import make_identity

F32 = mybir.dt.float32
F16 = mybir.dt.float16
I16 = mybir.dt.int16
AF = mybir.ActivationFunctionType
ALU = mybir.AluOpType
AX = mybir.AxisListType

V, D, OUT = 100000, 128, 2
H = 3 * D
B, L1, L2 = 4096, 19, 20
NWIN = 18
NCORES = 8
BC = B // NCORES          # 512 samples per core
NB = BC // 128            # 4 batch chunks of 128
NJ = 3                    # hidden segments of 128
VC = 20480                # compact per-core table rows (>= unique count)
N1 = L1 * BC              # e1 gather count
N2 = L2 * BC              # e2 gather count

_cache = {}


def _build():
    """Build (and cache) the Bass program shared by all 8 cores."""
    if "nc" in _cache:
        return _cache["nc"]

    nc = bacc.Bacc(
        "TRN2",
        target_bir_lowering=False,
        debug=False,
        enable_asserts=False,
        num_devices=NCORES,
        num_swdge_queues=4,
    )

    emb_d = nc.dram_tensor("emb", [VC, D], F16, kind="ExternalInput").ap()
    wt_d = nc.dram_tensor("wt", [4, 128, 4 * H], F16, kind="ExternalInput").ap()
    biasT_d = nc.dram_tensor("biasT", [1, 4 * H], F16, kind="ExternalInput").ap()
    lwb_d = nc.dram_tensor("lwb", [1, 4], F32, kind="ExternalInput").ap()
    ix1_d = nc.dram_tensor("ix1", [128, N1 // 16], I16, kind="ExternalInput").ap()
    ix2_d = nc.dram_tensor("ix2", [128, N2 // 16], I16, kind="ExternalInput").ap()
    out_d = nc.dram_tensor("out", [BC, OUT], F32, kind="ExternalOutput").ap()

    with tile.TileContext(nc) as tc, ExitStack() as ctx:
        singles = ctx.enter_context(tc.tile_pool(name="singles", bufs=1))
        psum_tr = ctx.enter_context(tc.tile_pool(name="psum_tr", bufs=1, space="PSUM"))
        psum_z = ctx.enter_context(tc.tile_pool(name="psum_z", bufs=2, space="PSUM"))
        gates = ctx.enter_context(tc.tile_pool(name="gates", bufs=6))
        hcpool = ctx.enter_context(tc.tile_pool(name="hc", bufs=2))
        tmp = ctx.enter_context(tc.tile_pool(name="tmp", bufs=3))
        prodp = ctx.enter_context(tc.tile_pool(name="prodp", bufs=3))
        small = ctx.enter_context(tc.tile_pool(name="small", bufs=2))

        # ---- constants (index tensors first: the gather stream waits on them) ----
        ix1_sb = singles.tile([128, N1 // 16], I16, tag="ix1")
        nc.sync.dma_start(out=ix1_sb[:], in_=ix1_d)
        ix2_sb = singles.tile([128, N2 // 16], I16, tag="ix2")
        nc.sync.dma_start(out=ix2_sb[:], in_=ix2_d)
        wt_sb = singles.tile([128, 4, 4 * H], F16, tag="wt")
        nc.sync.dma_start(out=wt_sb[:], in_=wt_d.rearrange("c p g -> p c g"))
        biasT_sb = singles.tile([1, 4 * H], F16, tag="biasT")
        nc.sync.dma_start(out=biasT_sb[:], in_=biasT_d)
        lwb_sb = singles.tile([128, 4], F32, tag="lwb")
        nc.sync.dma_start(out=lwb_sb[:], in_=lwb_d.to_broadcast([128, 4]))

        ones_sb = singles.tile([1, BC], F16, tag="ones")
        nc.vector.memset(ones_sb[:], 1.0)

        ident_f = singles.tile([128, 128], F32, tag="identf")
        make_identity(nc, ident_f[:])
        ident_h = singles.tile([128, 128], F16, tag="identh")
        nc.vector.tensor_copy(out=ident_h[:], in_=ident_f[:])

        xT = singles.tile([128, L1, BC], F16, tag="xT")
        g2 = singles.tile([128, NB * L2, D], F16, tag="g2")

        # ---- gather streams ----
        # e1: transpose-mode gathers write xT[d, w, b] directly. Queue 0 only
        # (transpose staging races across queues).
        for w in range(L1):
            out_ap = bass.AP(
                tensor=xT.tensor,
                offset=xT.offset + w * BC,
                ap=[xT.ap[0], [0, 1], [1, BC]],
            )
            nc.gpsimd.dma_gather(
                out_ap=out_ap,
                in_ap=emb_d,
                idxs_ap=ix1_sb[:, w * (BC // 16) : (w + 1) * (BC // 16)],
                num_idxs=BC,
                num_idxs_reg=BC,
                elem_size=D,
                transpose=True,
                single_packet=False,
                queue_num=0,
            )
        # e2: flat gathers, g2[p, cb*20+k, :] = emb[idx2[cb*128+p, k]]
        for cb in range(NB):
            nq = N2 // NB
            nc.gpsimd.dma_gather(
                out_ap=g2[:, cb * L2 : (cb + 1) * L2, :],
                in_ap=emb_d,
                idxs_ap=ix2_sb[:, cb * (nq // 16) : (cb + 1) * (nq // 16)],
                num_idxs=nq,
                num_idxs_reg=nq,
                elem_size=D,
                transpose=False,
                single_packet=False,
                queue_num=1 + (cb % 3),
            )

        # ---- LSTM steps ----
        # group order: o first (its ACT overlaps the c chain), then i, f, g
        QORDER = (3, 0, 1, 2)

        def emit_step(t, h_prev, c_prev):
            gq_t = [None] * 4
            for q in QORDER:
                zq = psum_z.tile([128, NJ, BC], F32, tag="z", name=f"z{t}_{q}")
                for j in range(NJ):
                    gc = q * NJ + j
                    cols = slice(gc * 128, (gc + 1) * 128)
                    nc.tensor.matmul(
                        out=zq[:, j, :],
                        lhsT=biasT_sb[0:1, cols],
                        rhs=ones_sb[0:1, :],
                        start=True,
                        stop=False,
                    )
                    nc.tensor.matmul(
                        out=zq[:, j, :],
                        lhsT=wt_sb[:, 0, cols],
                        rhs=xT[:, t, :],
                        start=False,
                        stop=(t == 0),
                    )
                    if t > 0:
                        for kj in range(NJ):
                            nc.tensor.matmul(
                                out=zq[:, j, :],
                                lhsT=wt_sb[:, 1 + kj, cols],
                                rhs=h_prev[:, kj, :],
                                start=False,
                                stop=(kj == NJ - 1),
                            )
                gq = gates.tile([128, NJ, BC], F16, tag=f"g{q}", name=f"g{t}_{q}")
                nc.scalar.activation(
                    out=gq[:],
                    in_=zq[:],
                    func=AF.Tanh if q == 2 else AF.Sigmoid,
                )
                gq_t[q] = gq

            gi, gf, gg, go = gq_t
            cn = hcpool.tile([128, NJ, BC], F16, tag="c", name=f"c{t}")
            for j in range(NJ):
                if t == 0:
                    nc.vector.tensor_tensor(
                        out=cn[:, j, :], in0=gi[:, j, :], in1=gg[:, j, :], op=ALU.mult
                    )
                else:
                    ig = tmp.tile([128, BC], F16, tag="ig", name=f"ig{t}_{j}")
                    nc.vector.tensor_tensor(
                        out=ig[:], in0=gi[:, j, :], in1=gg[:, j, :], op=ALU.mult
                    )
                    nc.vector.tensor_tensor(
                        out=cn[:, j, :],
                        in0=gf[:, j, :],
                        in1=c_prev[:, j, :],
                        op=ALU.mult,
                    )
                    nc.vector.tensor_tensor(
                        out=cn[:, j, :], in0=cn[:, j, :], in1=ig[:], op=ALU.add
                    )
            tc_t = tmp.tile([128, NJ, BC], F16, tag="tc", name=f"tc{t}")
            nc.scalar.activation(out=tc_t[:], in_=cn[:], func=AF.Tanh)
            hn = hcpool.tile([128, NJ, BC], F16, tag="h", name=f"h{t}")
            for j in range(NJ):
                nc.vector.tensor_tensor(
                    out=hn[:, j, :], in0=go[:, j, :], in1=tc_t[:, j, :], op=ALU.mult
                )
            return hn, cn

        h_t = None
        c_t = None
        for t in range(L1):
            h_t, c_t = emit_step(t, h_t, c_t)

        # ---- final h transposes: h_bmb[cb] = [128 b, 384] f16 ----
        h_bmb = [
            singles.tile([128, H], F16, tag=f"hbm{cb}", name=f"hbm{cb}")
            for cb in range(NB)
        ]
        for j in range(NJ):
            for cb in range(NB):
                ps = psum_tr.tile([128, 128], F16, tag="trps", name=f"htp{j}_{cb}")
                nc.tensor.transpose(
                    out=ps[:],
                    in_=h_t[:, j, cb * 128 : (cb + 1) * 128],
                    identity=ident_h[:],
                )
                nc.vector.tensor_copy(
                    out=h_bmb[cb][:, j * 128 : (j + 1) * 128], in_=ps[:]
                )

        # ---- window dots: s_j[b,k] = h_seg_j[b] . e2[b,k] ----
        s_t = []
        for cb in range(NB):
            s = small.tile([128, NJ, L2], F32, tag=f"s{cb}", name=f"s{cb}")
            s_t.append(s)
            g2cb = g2[:, cb * L2 : (cb + 1) * L2, :]
            for j in range(NJ):
                prod = prodp.tile([128, L2, D], F16, tag="prod", name=f"pr{cb}_{j}")
                hseg = h_bmb[cb][:, j * 128 : (j + 1) * 128]
                hbc = bass.AP(
                    tensor=hseg.tensor,
                    offset=hseg.offset,
                    ap=[hseg.ap[0], [0, L2], hseg.ap[1]],
                )
                # gpsimd (idle after gathers) helps with a few muls
                eng = nc.gpsimd if (j == 2 and cb < 2) else nc.vector
                eng.tensor_tensor(out=prod[:], in0=g2cb, in1=hbc, op=ALU.mult)
                # tree-sum: two f16 half-adds (2x mode), then a short reduce
                h1 = prodp.tile([128, L2, D // 2], F16, tag="half1",
                                name=f"h1_{cb}_{j}")
                nc.vector.tensor_tensor(
                    out=h1[:], in0=prod[:, :, 0 : D // 2],
                    in1=prod[:, :, D // 2 : D], op=ALU.add
                )
                h2 = prodp.tile([128, L2, D // 4], F16, tag="half2",
                                name=f"h2_{cb}_{j}")
                nc.vector.tensor_tensor(
                    out=h2[:], in0=h1[:, :, 0 : D // 4],
                    in1=h1[:, :, D // 4 : D // 2], op=ALU.add
                )
                nc.vector.tensor_reduce(
                    out=s[:, j, :], in_=h2[:], axis=AX.X, op=ALU.add
                )

        # ---- windows max + log-softmax (exp's batched, then ln's) ----
        a_t, negm_t, se_t, lse_t = [], [], [], []
        for cb in range(NB):
            s = s_t[cb]
            rs0 = small.tile([128, NWIN], F32, tag="rs0", name=f"rs0_{cb}")
            nc.vector.tensor_tensor(
                out=rs0[:], in0=s[:, 0, 0:NWIN], in1=s[:, 1, 1 : NWIN + 1], op=ALU.add
            )
            rs1 = small.tile([128, NWIN], F32, tag="rs1", name=f"rs1_{cb}")
            nc.vector.tensor_tensor(
                out=rs1[:], in0=rs0[:], in1=s[:, 2, 2 : NWIN + 2], op=ALU.add
            )
            ms = small.tile([128, 1], F32, tag="ms", name=f"ms{cb}")
            nc.vector.tensor_reduce(out=ms[:], in_=rs1[:], axis=AX.X, op=ALU.max)
            a = small.tile([128, OUT], F32, tag=f"a{cb}", name=f"a{cb}")
            nc.vector.scalar_tensor_tensor(
                out=a[:],
                in0=lwb_sb[:, 0:2],
                scalar=ms[:, 0:1],
                in1=lwb_sb[:, 2:4],
                op0=ALU.mult,
                op1=ALU.add,
            )
            negm = small.tile([128, 1], F32, tag=f"negm{cb}", name=f"negm{cb}")
            nc.vector.tensor_reduce(
                out=negm[:], in_=a[:], axis=AX.X, op=ALU.max, negate=True
            )
            a_t.append(a)
            negm_t.append(negm)
        for cb in range(NB):
            e = small.tile([128, OUT], F32, tag=f"e{cb}", name=f"e{cb}")
            se = small.tile([128, 1], F32, tag=f"se{cb}", name=f"se{cb}")
            nc.scalar.activation(
                out=e[:], in_=a_t[cb][:], func=AF.Exp, bias=negm_t[cb][:, 0:1],
                accum_out=se[:],
            )
            se_t.append(se)
        for cb in range(NB):
            lse = small.tile([128, 1], F32, tag=f"lse{cb}", name=f"lse{cb}")
            nc.scalar.activation(out=lse[:], in_=se_t[cb][:], func=AF.Ln)
            lse_t.append(lse)
        for cb in range(NB):
            combo = small.tile([128, 1], F32, tag=f"combo{cb}", name=f"combo{cb}")
            nc.vector.tensor_tensor(
                out=combo[:], in0=negm_t[cb][:], in1=lse_t[cb][:], op=ALU.subtract
            )
            ot = small.tile([128, OUT], F32, tag=f"ot{cb}", name=f"ot{cb}")
            nc.vector.tensor_scalar_add(ot[:], a_t[cb][:], combo[:, 0:1])
            nc.sync.dma_start(out=out_d[cb * 128 : (cb + 1) * 128, :], in_=ot[:])

    nc.compile()
    _cache["nc"] = nc
    return nc


def _wrap16(flat):
    """idx i -> partition i%16, col i//16; replicated to 128 partitions."""
    n = len(flat)
    a = np.zeros((16, n // 16), np.int16)
    a[np.arange(n) % 16, np.arange(n) // 16] = flat.astype(np.int16)
    return np.tile(a, (8, 1))


def kernel(input1, input2, emb, W_ih, W_hh, b_ih, b_hh, lin_w, lin_b, _trace=False):
    input1 = np.ascontiguousarray(np.asarray(input1, dtype=np.int64))
    input2 = np.ascontiguousarray(np.asarray(input2, dtype=np.int64))
    emb = np.asarray(emb, dtype=np.float32)
    W_ih = np.asarray(W_ih, dtype=np.float32)
    W_hh = np.asarray(W_hh, dtype=np.float32)
    b = (np.asarray(b_ih, dtype=np.float32) + np.asarray(b_hh, dtype=np.float32))
    lin_w = np.asarray(lin_w, dtype=np.float32)
    lin_b = np.asarray(lin_b, dtype=np.float32)

    # weight layout: Wfull = [W_ih | W_hh] : [4H, 128+384]; lhsT tiles are
    # Wfull.T reshaped [4 k-chunks, 128, 4H]
    wfull = np.concatenate([W_ih, W_hh], axis=1)          # [1536, 512]
    wt = np.ascontiguousarray(wfull.T.reshape(4, 128, 4 * H).astype(np.float16))
    biasT = np.ascontiguousarray(b.reshape(1, 4 * H).astype(np.float16))
    lwb = np.ascontiguousarray(
        np.array([[lin_w[0, 0], lin_w[1, 0], lin_b[0], lin_b[1]]], dtype=np.float32)
    )

    nc = _build()

    in_maps = []
    for c in range(NCORES):
        i1 = input1[c * BC : (c + 1) * BC]                # [512, 19]
        i2 = input2[c * BC : (c + 1) * BC]                # [512, 20]
        ids = np.unique(np.concatenate([i1.ravel(), i2.ravel()]))
        assert len(ids) <= VC, len(ids)
        embc = np.zeros((VC, D), np.float16)
        embc[: len(ids)] = emb[ids].astype(np.float16)
        c1 = np.searchsorted(ids, i1)                     # [512, 19]
        c2 = np.searchsorted(ids, i2)                     # [512, 20]
        # e1 flat order per wave: i = b (= cb*128+p)
        ix1 = np.concatenate(
            [_wrap16(c1[:, w]) for w in range(L1)], axis=1
        )
        # e2 flat order per cb: i = p + 128*k
        ix2 = np.concatenate(
            [_wrap16(c2[cb * 128 : (cb + 1) * 128].T.ravel()) for cb in range(NB)],
            axis=1,
        )
        in_maps.append(
            {
                "emb": embc,
                "wt": wt,
                "biasT": biasT,
                "lwb": lwb,
                "ix1": np.ascontiguousarray(ix1),
                "ix2": np.ascontiguousarray(ix2),
            }
        )

    res = bass_utils.run_bass_kernel_spmd(
        nc, in_maps, core_ids=list(range(NCORES)), trace=_trace
    )
    if _trace:
        kernel.last_results = res
    out = np.concatenate([res.results[c]["out"] for c in range(NCORES)], axis=0)
    return out


if __name__ == "__main__":
    rng = np.random.default_rng(0)
    inputs = {
        "input1": rng.integers(0, V, (B, L1), dtype=np.int32),
        "input2": rng.integers(0, V, (B, L2), dtype=np.int32),
        "emb": rng.standard_normal((V, D), dtype=np.float32),
        "W_ih": (rng.standard_normal((4 * H, D), dtype=np.float32) * 0.05),
        "W_hh": (rng.standard_normal((4 * H, H), dtype=np.float32) * 0.05),
        "b_ih": (rng.standard_normal(4 * H).astype(np.float32) * 0.05),
        "b_hh": (rng.standard_normal(4 * H).astype(np.float32) * 0.05),
        "lin_w": rng.standard_normal((OUT, 1), dtype=np.float32),
        "lin_b": rng.standard_normal(OUT).astype(np.float32),
    }
    out = kernel(**inputs)
    print(out.shape, out[:2])


# revision 7
# speedup vs baseline: 2.1752x; 2.1752x over previous
"""Trainium2 Bass kernel for nn_LookupTableMy (embedding gathers + LSTM + window dots).

Computation (per sample b):
  e1 = emb[input1[b]]                 # [19, 128]
  h  = LSTM(e1)  (H=384, 19 steps)    # final hidden [384]
  e2 = emb[input2[b]]                 # [20, 128]
  s_j[k] = h[128j:128j+128] . e2[k]   # j=0..2, k=0..19
  rs[n]  = s_0[n] + s_1[n+1] + s_2[n+2]   n=0..17
  ms     = max_n rs[n]
  out    = log_softmax(ms * lin_w[:,0] + lin_b)   # [2]

Sharding: data-parallel over batch: 4096 samples -> 8 cores x 512.
Weights replicated; the embedding table is compacted per core on the host
(each core touches <= 19,968 of the 100k rows; unique rows are packed into
a [20480, 128] f16 table and indices remapped to int16).

v3 design (vs the 426us f32r baseline):
  - dma_gather (gpsimd SWDGE, mlp library) instead of 156 single-row
    indirect DMAs: e1 is 19 transpose-mode gathers (one per timestep,
    512 rows each) that write xT[d, t, b] DIRECTLY -- no PE transposes,
    no DVE copies. e2 is 4 flat gathers (2560 rows each). Transpose-mode
    gathers race across SWDGE queues (shared staging), so e1 stays on
    queue 0; flat e2 gathers spread over queues 1-3.
  - All matmul operands f16 (weights, x, h): rhs streams 2 elem/cycle,
    ~131 ns/MM at N=512 vs ~233 ns for f32r.
  - Gate-major z chunks [128 gate-dims, 512 batch] accumulate in PSUM in
    4 per-gate groups of 3 banks. Bias is pre-seeded into PSUM by a K=1
    matmul (biasT x ones-row), letting one ACTIVATE cover all 3 segments
    of a gate ([128, 1536]): ACT drops from 15x720ns to ~5x1573ns/step.
    o-gate group first so its ACT overlaps the c-path chain.
  - tanh(c) merged over segments; c/h single [128, 3, 512] f16 tiles
    (h slices feed next step's matmuls directly).
"""

import sys
from contextlib import ExitStack

for _p in ("/opt/trn_rl_repo",):
    if _p not in sys.path:
        sys.path.insert(0, _p)

import numpy as np

import concourse.bass as bass
import concourse.tile as tile
import concourse.bacc as bacc
import concourse.mybir as mybir
from concourse import bass_utils
from concourse.masks import make_identity

F32 = mybir.dt.float32
F16 = mybir.dt.float16
I16 = mybir.dt.int16
AF = mybir.ActivationFunctionType
ALU = mybir.AluOpType
AX = mybir.AxisListType

V, D, OUT = 100000, 128, 2
H = 3 * D
B, L1, L2 = 4096, 19, 20
NWIN = 18
NCORES = 8
BC = B // NCORES          # 512 samples per core
NB = BC // 128            # 4 batch chunks of 128
NJ = 3                    # hidden segments of 128
VC = 20480                # compact per-core table rows (>= unique count)
N1 = L1 * BC              # e1 gather count
N2 = L2 * BC              # e2 gather count

_cache = {}


def _build():
    """Build (and cache) the Bass program shared by all 8 cores."""
    if "nc" in _cache:
        return _cache["nc"]

    nc = bacc.Bacc(
        "TRN2",
        target_bir_lowering=False,
        debug=False,
        enable_asserts=False,
        num_devices=NCORES,
        num_swdge_queues=4,
    )

    emb_d = nc.dram_tensor("emb", [VC, D], F16, kind="ExternalInput").ap()
    wt_d = nc.dram_tensor("wt", [4, 128, 4 * H], F16, kind="ExternalInput").ap()
    biasc_d = nc.dram_tensor("biasc", [4 * H // 128, 128], F32, kind="ExternalInput").ap()
    lwb_d = nc.dram_tensor("lwb", [1, 4], F32, kind="ExternalInput").ap()
    ix1_d = nc.dram_tensor("ix1", [128, N1 // 16], I16, kind="ExternalInput").ap()
    ix2_d = nc.dram_tensor("ix2", [128, N2 // 16], I16, kind="ExternalInput").ap()
    out_d = nc.dram_tensor("out", [BC, OUT], F32, kind="ExternalOutput").ap()

    with tile.TileContext(nc) as tc, ExitStack() as ctx:
        singles = ctx.enter_context(tc.tile_pool(name="singles", bufs=1))
        psum_tr = ctx.enter_context(tc.tile_pool(name="psum_tr", bufs=1, space="PSUM"))
        psum_z = ctx.enter_context(tc.tile_pool(name="psum_z", bufs=7, space="PSUM"))
        gates = ctx.enter_context(tc.tile_pool(name="gates", bufs=2))
        hcpool = ctx.enter_context(tc.tile_pool(name="hc", bufs=2))
        tmp = ctx.enter_context(tc.tile_pool(name="tmp", bufs=3))
        prodp = ctx.enter_context(tc.tile_pool(name="prodp", bufs=3))
        small = ctx.enter_context(tc.tile_pool(name="small", bufs=2))

        # ---- constants (index tensors first: the gather stream waits on them) ----
        ix1_sb = singles.tile([128, N1 // 16], I16, tag="ix1")
        nc.sync.dma_start(out=ix1_sb[:], in_=ix1_d)
        ix2_sb = singles.tile([128, N2 // 16], I16, tag="ix2")
        nc.sync.dma_start(out=ix2_sb[:], in_=ix2_d)
        wt_sb = singles.tile([128, 4, 4 * H], F16, tag="wt")
        nc.sync.dma_start(out=wt_sb[:], in_=wt_d.rearrange("c p g -> p c g"))
        bias_col = singles.tile([128, 4 * H // 128], F32, tag="biascol")
        nc.sync.dma_start(out=bias_col[:], in_=biasc_d.rearrange("g p -> p g"))
        lwb_sb = singles.tile([128, 4], F32, tag="lwb")
        nc.sync.dma_start(out=lwb_sb[:], in_=lwb_d.to_broadcast([128, 4]))

        ident_f = singles.tile([128, 128], F32, tag="identf")
        make_identity(nc, ident_f[:])
        ident_h = singles.tile([128, 128], F16, tag="identh")
        nc.vector.tensor_copy(out=ident_h[:], in_=ident_f[:])

        xT = singles.tile([128, L1, BC], F16, tag="xT")
        g2 = singles.tile([128, NB * L2, D], F16, tag="g2")

        # ---- gather streams ----
        # e1: transpose-mode gathers write xT[d, w, b] directly. Queue 0 only
        # (transpose staging races across queues).
        for w in range(L1):
            out_ap = bass.AP(
                tensor=xT.tensor,
                offset=xT.offset + w * BC,
                ap=[xT.ap[0], [0, 1], [1, BC]],
            )
            nc.gpsimd.dma_gather(
                out_ap=out_ap,
                in_ap=emb_d,
                idxs_ap=ix1_sb[:, w * (BC // 16) : (w + 1) * (BC // 16)],
                num_idxs=BC,
                num_idxs_reg=BC,
                elem_size=D,
                transpose=True,
                single_packet=False,
                queue_num=0,
            )
        # e2: flat gathers, g2[p, cb*20+k, :] = emb[idx2[cb*128+p, k]]
        for cb in range(NB):
            nq = N2 // NB
            nc.gpsimd.dma_gather(
                out_ap=g2[:, cb * L2 : (cb + 1) * L2, :],
                in_ap=emb_d,
                idxs_ap=ix2_sb[:, cb * (nq // 16) : (cb + 1) * (nq // 16)],
                num_idxs=nq,
                num_idxs_reg=nq,
                elem_size=D,
                transpose=False,
                single_packet=False,
                queue_num=1 + (cb % 3),
            )

        # ---- LSTM steps ----
        # Per-chunk z psum tiles (1 bank each, deep pipelining). Chunk order
        # (i_j, f_j, g_j) per segment then the o chunks: c_j starts as soon as
        # segment j's three gates are done while ACT continues with o; the
        # o ACTs and tanh(c) overlap the next step's x-matmuls.
        CHUNKS = [(q, j) for j in range(NJ) for q in (0, 1, 2)] + [
            (3, j) for j in range(NJ)
        ]

        def emit_step(t, h_prev, c_prev):
            gq = {}
            for q, j in CHUNKS:
                gc = q * NJ + j
                cols = slice(gc * 128, (gc + 1) * 128)
                zq = psum_z.tile([128, BC], F32, tag="z", name=f"z{t}_{gc}")
                nc.tensor.matmul(
                    out=zq[:],
                    lhsT=wt_sb[:, 0, cols],
                    rhs=xT[:, t, :],
                    start=True,
                    stop=(t == 0),
                )
                if t > 0:
                    for kj in range(NJ):
                        nc.tensor.matmul(
                            out=zq[:],
                            lhsT=wt_sb[:, 1 + kj, cols],
                            rhs=h_prev[:, kj, :],
                            start=False,
                            stop=(kj == NJ - 1),
                        )
                g = gates.tile([128, BC], F16, tag=f"g{gc}", name=f"g{t}_{gc}")
                nc.scalar.activation(
                    out=g[:],
                    in_=zq[:],
                    func=AF.Tanh if q == 2 else AF.Sigmoid,
                    bias=bias_col[:, gc : gc + 1],
                )
                gq[(q, j)] = g

            cn = hcpool.tile([128, NJ, BC], F16, tag="c", name=f"c{t}")
            for j in range(NJ):
                gi, gf, gg = gq[(0, j)], gq[(1, j)], gq[(2, j)]
                if t == 0:
                    nc.vector.tensor_tensor(
                        out=cn[:, j, :], in0=gi[:], in1=gg[:], op=ALU.mult
                    )
                else:
                    ig = tmp.tile([128, BC], F16, tag="ig", name=f"ig{t}_{j}")
                    nc.vector.tensor_tensor(
                        out=ig[:], in0=gi[:], in1=gg[:], op=ALU.mult
                    )
                    nc.vector.tensor_tensor(
                        out=cn[:, j, :],
                        in0=gf[:],
                        in1=c_prev[:, j, :],
                        op=ALU.mult,
                    )
                    nc.vector.tensor_tensor(
                        out=cn[:, j, :], in0=cn[:, j, :], in1=ig[:], op=ALU.add
                    )
            tc_t = tmp.tile([128, NJ, BC], F16, tag="tc", name=f"tc{t}")
            nc.scalar.activation(out=tc_t[:], in_=cn[:], func=AF.Tanh)
            hn = hcpool.tile([128, NJ, BC], F16, tag="h", name=f"h{t}")
            for j in range(NJ):
                nc.vector.tensor_tensor(
                    out=hn[:, j, :], in0=gq[(3, j)][:], in1=tc_t[:, j, :],
                    op=ALU.mult
                )
            return hn, cn

        h_t = None
        c_t = None
        for t in range(L1):
            h_t, c_t = emit_step(t, h_t, c_t)

        # ---- final h transposes: h_bmb[cb] = [128 b, 384] f16 ----
        h_bmb = [
            singles.tile([128, H], F16, tag=f"hbm{cb}", name=f"hbm{cb}")
            for cb in range(NB)
        ]
        for j in range(NJ):
            for cb in range(NB):
                ps = psum_tr.tile([128, 128], F16, tag="trps", name=f"htp{j}_{cb}")
                nc.tensor.transpose(
                    out=ps[:],
                    in_=h_t[:, j, cb * 128 : (cb + 1) * 128],
                    identity=ident_h[:],
                )
                nc.vector.tensor_copy(
                    out=h_bmb[cb][:, j * 128 : (j + 1) * 128], in_=ps[:]
                )

        # ---- window dots: s_j[b,k] = h_seg_j[b] . e2[b,k] ----
        s_t = []
        for cb in range(NB):
            s = small.tile([128, NJ, L2], F32, tag=f"s{cb}", name=f"s{cb}")
            s_t.append(s)
            g2cb = g2[:, cb * L2 : (cb + 1) * L2, :]
            for j in range(NJ):
                prod = prodp.tile([128, L2, D], F16, tag="prod", name=f"pr{cb}_{j}")
                hseg = h_bmb[cb][:, j * 128 : (j + 1) * 128]
                hbc = bass.AP(
                    tensor=hseg.tensor,
                    offset=hseg.offset,
                    ap=[hseg.ap[0], [0, L2], hseg.ap[1]],
                )
                # gpsimd (idle after gathers) helps with a few muls
                eng = nc.gpsimd if (j == 2 and cb < 2) else nc.vector
                eng.tensor_tensor(out=prod[:], in0=g2cb, in1=hbc, op=ALU.mult)
                # tree-sum: two f16 half-adds (2x mode), then a short reduce
                h1 = prodp.tile([128, L2, D // 2], F16, tag="half1",
                                name=f"h1_{cb}_{j}")
                nc.vector.tensor_tensor(
                    out=h1[:], in0=prod[:, :, 0 : D // 2],
                    in1=prod[:, :, D // 2 : D], op=ALU.add
                )
                h2 = prodp.tile([128, L2, D // 4], F16, tag="half2",
                                name=f"h2_{cb}_{j}")
                nc.vector.tensor_tensor(
                    out=h2[:], in0=h1[:, :, 0 : D // 4],
                    in1=h1[:, :, D // 4 : D // 2], op=ALU.add
                )
                nc.vector.tensor_reduce(
                    out=s[:, j, :], in_=h2[:], axis=AX.X, op=ALU.add
                )

        # ---- windows max + log-softmax (exp's batched, then ln's) ----
        a_t, negm_t, se_t, lse_t = [], [], [], []
        for cb in range(NB):
            s = s_t[cb]
            rs0 = small.tile([128, NWIN], F32, tag="rs0", name=f"rs0_{cb}")
            nc.vector.tensor_tensor(
                out=rs0[:], in0=s[:, 0, 0:NWIN], in1=s[:, 1, 1 : NWIN + 1], op=ALU.add
            )
            rs1 = small.tile([128, NWIN], F32, tag="rs1", name=f"rs1_{cb}")
            nc.vector.tensor_tensor(
                out=rs1[:], in0=rs0[:], in1=s[:, 2, 2 : NWIN + 2], op=ALU.add
            )
            ms = small.tile([128, 1], F32, tag="ms", name=f"ms{cb}")
            nc.vector.tensor_reduce(out=ms[:], in_=rs1[:], axis=AX.X, op=ALU.max)
            a = small.tile([128, OUT], F32, tag=f"a{cb}", name=f"a{cb}")
            nc.vector.scalar_tensor_tensor(
                out=a[:],
                in0=lwb_sb[:, 0:2],
                scalar=ms[:, 0:1],
                in1=lwb_sb[:, 2:4],
                op0=ALU.mult,
                op1=ALU.add,
            )
            negm = small.tile([128, 1], F32, tag=f"negm{cb}", name=f"negm{cb}")
            nc.vector.tensor_reduce(
                out=negm[:], in_=a[:], axis=AX.X, op=ALU.max, negate=True
            )
            a_t.append(a)
            negm_t.append(negm)
        for cb in range(NB):
            e = small.tile([128, OUT], F32, tag=f"e{cb}", name=f"e{cb}")
            se = small.tile([128, 1], F32, tag=f"se{cb}", name=f"se{cb}")
            nc.scalar.activation(
                out=e[:], in_=a_t[cb][:], func=AF.Exp, bias=negm_t[cb][:, 0:1],
                accum_out=se[:],
            )
            se_t.append(se)
        for cb in range(NB):
            lse = small.tile([128, 1], F32, tag=f"lse{cb}", name=f"lse{cb}")
            nc.scalar.activation(out=lse[:], in_=se_t[cb][:], func=AF.Ln)
            lse_t.append(lse)
        for cb in range(NB):
            combo = small.tile([128, 1], F32, tag=f"combo{cb}", name=f"combo{cb}")
            nc.vector.tensor_tensor(
                out=combo[:], in0=negm_t[cb][:], in1=lse_t[cb][:], op=ALU.subtract
            )
            ot = small.tile([128, OUT], F32, tag=f"ot{cb}", name=f"ot{cb}")
            nc.vector.tensor_scalar_add(ot[:], a_t[cb][:], combo[:, 0:1])
            nc.sync.dma_start(out=out_d[cb * 128 : (cb + 1) * 128, :], in_=ot[:])

    nc.compile()
    _cache["nc"] = nc
    return nc


def _wrap16(flat):
    """idx i -> partition i%16, col i//16; replicated to 128 partitions."""
    n = len(flat)
    a = np.zeros((16, n // 16), np.int16)
    a[np.arange(n) % 16, np.arange(n) // 16] = flat.astype(np.int16)
    return np.tile(a, (8, 1))


def kernel(input1, input2, emb, W_ih, W_hh, b_ih, b_hh, lin_w, lin_b, _trace=False):
    input1 = np.ascontiguousarray(np.asarray(input1, dtype=np.int64))
    input2 = np.ascontiguousarray(np.asarray(input2, dtype=np.int64))
    emb = np.asarray(emb, dtype=np.float32)
    W_ih = np.asarray(W_ih, dtype=np.float32)
    W_hh = np.asarray(W_hh, dtype=np.float32)
    b = (np.asarray(b_ih, dtype=np.float32) + np.asarray(b_hh, dtype=np.float32))
    lin_w = np.asarray(lin_w, dtype=np.float32)
    lin_b = np.asarray(lin_b, dtype=np.float32)

    # weight layout: Wfull = [W_ih | W_hh] : [4H, 128+384]; lhsT tiles are
    # Wfull.T reshaped [4 k-chunks, 128, 4H]
    wfull = np.concatenate([W_ih, W_hh], axis=1)          # [1536, 512]
    wt = np.ascontiguousarray(wfull.T.reshape(4, 128, 4 * H).astype(np.float16))
    biasc = np.ascontiguousarray(b.reshape(4 * H // 128, 128))
    lwb = np.ascontiguousarray(
        np.array([[lin_w[0, 0], lin_w[1, 0], lin_b[0], lin_b[1]]], dtype=np.float32)
    )

    nc = _build()

    in_maps = []
    for c in range(NCORES):
        i1 = input1[c * BC : (c + 1) * BC]                # [512, 19]
        i2 = input2[c * BC : (c + 1) * BC]                # [512, 20]
        ids = np.unique(np.concatenate([i1.ravel(), i2.ravel()]))
        assert len(ids) <= VC, len(ids)
        embc = np.zeros((VC, D), np.float16)
        embc[: len(ids)] = emb[ids].astype(np.float16)
        c1 = np.searchsorted(ids, i1)                     # [512, 19]
        c2 = np.searchsorted(ids, i2)                     # [512, 20]
        # e1 flat order per wave: i = b (= cb*128+p)
        ix1 = np.concatenate(
            [_wrap16(c1[:, w]) for w in range(L1)], axis=1
        )
        # e2 flat order per cb: i = p + 128*k
        ix2 = np.concatenate(
            [_wrap16(c2[cb * 128 : (cb + 1) * 128].T.ravel()) for cb in range(NB)],
            axis=1,
        )
        in_maps.append(
            {
                "emb": embc,
                "wt": wt,
                "biasc": biasc,
                "lwb": lwb,
                "ix1": np.ascontiguousarray(ix1),
                "ix2": np.ascontiguousarray(ix2),
            }
        )

    res = bass_utils.run_bass_kernel_spmd(
        nc, in_maps, core_ids=list(range(NCORES)), trace=_trace
    )
    if _trace:
        kernel.last_results = res
    out = np.concatenate([res.results[c]["out"] for c in range(NCORES)], axis=0)
    return out


if __name__ == "__main__":
    rng = np.random.default_rng(0)
    inputs = {
        "input1": rng.integers(0, V, (B, L1), dtype=np.int32),
        "input2": rng.integers(0, V, (B, L2), dtype=np.int32),
        "emb": rng.standard_normal((V, D), dtype=np.float32),
        "W_ih": (rng.standard_normal((4 * H, D), dtype=np.float32) * 0.05),
        "W_hh": (rng.standard_normal((4 * H, H), dtype=np.float32) * 0.05),
        "b_ih": (rng.standard_normal(4 * H).astype(np.float32) * 0.05),
        "b_hh": (rng.standard_normal(4 * H).astype(np.float32) * 0.05),
        "lin_w": rng.standard_normal((OUT, 1), dtype=np.float32),
        "lin_b": rng.standard_normal(OUT).astype(np.float32),
    }
    out = kernel(**inputs)
    print(out.shape, out[:2])


# revision 8
# speedup vs baseline: 2.2215x; 1.0213x over previous
"""Trainium2 Bass kernel for nn_LookupTableMy (embedding gathers + LSTM + window dots).

Computation (per sample b):
  e1 = emb[input1[b]]                 # [19, 128]
  h  = LSTM(e1)  (H=384, 19 steps)    # final hidden [384]
  e2 = emb[input2[b]]                 # [20, 128]
  s_j[k] = h[128j:128j+128] . e2[k]   # j=0..2, k=0..19
  rs[n]  = s_0[n] + s_1[n+1] + s_2[n+2]   n=0..17
  ms     = max_n rs[n]
  out    = log_softmax(ms * lin_w[:,0] + lin_b)   # [2]

Sharding: data-parallel over batch: 4096 samples -> 8 cores x 512.
Weights replicated; the embedding table is compacted per core on the host
(each core touches <= 19,968 of the 100k rows; unique rows are packed into
a [20480, 128] f16 table and indices remapped to int16).

v3 design (vs the 426us f32r baseline):
  - dma_gather (gpsimd SWDGE, mlp library) instead of 156 single-row
    indirect DMAs: e1 is 19 transpose-mode gathers (one per timestep,
    512 rows each) that write xT[d, t, b] DIRECTLY -- no PE transposes,
    no DVE copies. e2 is 4 flat gathers (2560 rows each). Transpose-mode
    gathers race across SWDGE queues (shared staging), so e1 stays on
    queue 0; flat e2 gathers spread over queues 1-3.
  - All matmul operands f16 (weights, x, h): rhs streams 2 elem/cycle,
    ~131 ns/MM at N=512 vs ~233 ns for f32r.
  - Gate-major z chunks [128 gate-dims, 512 batch] accumulate in PSUM in
    4 per-gate groups of 3 banks. Bias is pre-seeded into PSUM by a K=1
    matmul (biasT x ones-row), letting one ACTIVATE cover all 3 segments
    of a gate ([128, 1536]): ACT drops from 15x720ns to ~5x1573ns/step.
    o-gate group first so its ACT overlaps the c-path chain.
  - tanh(c) merged over segments; c/h single [128, 3, 512] f16 tiles
    (h slices feed next step's matmuls directly).
"""

import sys
from contextlib import ExitStack

for _p in ("/opt/trn_rl_repo",):
    if _p not in sys.path:
        sys.path.insert(0, _p)

import numpy as np

import concourse.bass as bass
import concourse.tile as tile
import concourse.bacc as bacc
import concourse.mybir as mybir
from concourse import bass_utils

F32 = mybir.dt.float32
F16 = mybir.dt.float16
I16 = mybir.dt.int16
AF = mybir.ActivationFunctionType
ALU = mybir.AluOpType
AX = mybir.AxisListType

V, D, OUT = 100000, 128, 2
H = 3 * D
B, L1, L2 = 4096, 19, 20
NWIN = 18
NCORES = 8
BC = B // NCORES          # 512 samples per core
NB = BC // 128            # 4 batch chunks of 128
NJ = 3                    # hidden segments of 128
VC = 20480                # compact per-core table rows (>= unique count)
N1 = L1 * BC              # e1 gather count
N2 = L2 * BC              # e2 gather count

_cache = {}


def _build():
    """Build (and cache) the Bass program shared by all 8 cores."""
    if "nc" in _cache:
        return _cache["nc"]

    nc = bacc.Bacc(
        "TRN2",
        target_bir_lowering=False,
        debug=False,
        enable_asserts=False,
        num_devices=NCORES,
        num_swdge_queues=4,
    )

    emb_d = nc.dram_tensor("emb", [VC, D], F16, kind="ExternalInput").ap()
    wt_d = nc.dram_tensor("wt", [4, 128, 4 * H], F16, kind="ExternalInput").ap()
    biasc_d = nc.dram_tensor("biasc", [4 * H // 128, 128], F32, kind="ExternalInput").ap()
    lwb_d = nc.dram_tensor("lwb", [1, 4], F32, kind="ExternalInput").ap()
    ident_d = nc.dram_tensor("ident", [128, 128], F16, kind="ExternalInput").ap()
    ix1_d = nc.dram_tensor("ix1", [128, N1 // 16], I16, kind="ExternalInput").ap()
    ix2_d = nc.dram_tensor("ix2", [128, N2 // 16], I16, kind="ExternalInput").ap()
    out_d = nc.dram_tensor("out", [BC, OUT], F32, kind="ExternalOutput").ap()

    with tile.TileContext(nc) as tc, ExitStack() as ctx:
        singles = ctx.enter_context(tc.tile_pool(name="singles", bufs=1))
        psum_tr = ctx.enter_context(tc.tile_pool(name="psum_tr", bufs=1, space="PSUM"))
        psum_z = ctx.enter_context(tc.tile_pool(name="psum_z", bufs=7, space="PSUM"))
        gates = ctx.enter_context(tc.tile_pool(name="gates", bufs=2))
        hcpool = ctx.enter_context(tc.tile_pool(name="hc", bufs=2))
        tmp = ctx.enter_context(tc.tile_pool(name="tmp", bufs=3))
        prodp = ctx.enter_context(tc.tile_pool(name="prodp", bufs=3))
        small = ctx.enter_context(tc.tile_pool(name="small", bufs=2))

        # ---- constants (index tensors first: the gather stream waits on them) ----
        ix1_sb = singles.tile([128, N1 // 16], I16, tag="ix1")
        nc.sync.dma_start(out=ix1_sb[:], in_=ix1_d)
        ix2_sb = singles.tile([128, N2 // 16], I16, tag="ix2")
        nc.sync.dma_start(out=ix2_sb[:], in_=ix2_d)
        wt_sb = singles.tile([128, 4, 4 * H], F16, tag="wt")
        nc.sync.dma_start(out=wt_sb[:], in_=wt_d.rearrange("c p g -> p c g"))
        bias_col = singles.tile([128, 4 * H // 128], F32, tag="biascol")
        nc.sync.dma_start(out=bias_col[:], in_=biasc_d.rearrange("g p -> p g"))
        lwb_sb = singles.tile([128, 4], F32, tag="lwb")
        nc.sync.dma_start(out=lwb_sb[:], in_=lwb_d.to_broadcast([128, 4]))

        ident_h = singles.tile([128, 128], F16, tag="identh")
        nc.sync.dma_start(out=ident_h[:], in_=ident_d)

        xT = singles.tile([128, L1, BC], F16, tag="xT")
        g2 = singles.tile([128, NB * L2, D], F16, tag="g2")

        # ---- gather streams ----
        # e1: transpose-mode gathers write xT[d, w, b] directly. Queue 0 only
        # (transpose staging races across queues).
        for w in range(L1):
            out_ap = bass.AP(
                tensor=xT.tensor,
                offset=xT.offset + w * BC,
                ap=[xT.ap[0], [0, 1], [1, BC]],
            )
            nc.gpsimd.dma_gather(
                out_ap=out_ap,
                in_ap=emb_d,
                idxs_ap=ix1_sb[:, w * (BC // 16) : (w + 1) * (BC // 16)],
                num_idxs=BC,
                num_idxs_reg=BC,
                elem_size=D,
                transpose=True,
                single_packet=False,
                queue_num=0,
            )
        # e2: flat gathers, g2[p, cb*20+k, :] = emb[idx2[cb*128+p, k]]
        for cb in range(NB):
            nq = N2 // NB
            nc.gpsimd.dma_gather(
                out_ap=g2[:, cb * L2 : (cb + 1) * L2, :],
                in_ap=emb_d,
                idxs_ap=ix2_sb[:, cb * (nq // 16) : (cb + 1) * (nq // 16)],
                num_idxs=nq,
                num_idxs_reg=nq,
                elem_size=D,
                transpose=False,
                single_packet=False,
                queue_num=1 + (cb % 3),
            )

        # ---- LSTM steps ----
        # Per-chunk z psum tiles (1 bank each, deep pipelining). Chunk order
        # (i_j, f_j, g_j) per segment then the o chunks: c_j starts as soon as
        # segment j's three gates are done while ACT continues with o; the
        # o ACTs and tanh(c) overlap the next step's x-matmuls.
        CHUNKS = [(q, j) for j in range(NJ) for q in (0, 1, 2)] + [
            (3, j) for j in range(NJ)
        ]

        def emit_step(t, h_prev, c_prev):
            gq = {}
            for q, j in CHUNKS:
                gc = q * NJ + j
                cols = slice(gc * 128, (gc + 1) * 128)
                zq = psum_z.tile([128, BC], F32, tag="z", name=f"z{t}_{gc}")
                nc.tensor.matmul(
                    out=zq[:],
                    lhsT=wt_sb[:, 0, cols],
                    rhs=xT[:, t, :],
                    start=True,
                    stop=(t == 0),
                )
                if t > 0:
                    for kj in range(NJ):
                        nc.tensor.matmul(
                            out=zq[:],
                            lhsT=wt_sb[:, 1 + kj, cols],
                            rhs=h_prev[:, kj, :],
                            start=False,
                            stop=(kj == NJ - 1),
                        )
                g = gates.tile([128, BC], F16, tag=f"g{gc}", name=f"g{t}_{gc}")
                nc.scalar.activation(
                    out=g[:],
                    in_=zq[:],
                    func=AF.Tanh if q == 2 else AF.Sigmoid,
                    bias=bias_col[:, gc : gc + 1],
                )
                gq[(q, j)] = g

            cn = hcpool.tile([128, NJ, BC], F16, tag="c", name=f"c{t}")
            for j in range(NJ):
                gi, gf, gg = gq[(0, j)], gq[(1, j)], gq[(2, j)]
                if t == 0:
                    nc.vector.tensor_tensor(
                        out=cn[:, j, :], in0=gi[:], in1=gg[:], op=ALU.mult
                    )
                else:
                    ig = tmp.tile([128, BC], F16, tag="ig", name=f"ig{t}_{j}")
                    nc.vector.tensor_tensor(
                        out=ig[:], in0=gi[:], in1=gg[:], op=ALU.mult
                    )
                    nc.vector.tensor_tensor(
                        out=cn[:, j, :],
                        in0=gf[:],
                        in1=c_prev[:, j, :],
                        op=ALU.mult,
                    )
                    nc.vector.tensor_tensor(
                        out=cn[:, j, :], in0=cn[:, j, :], in1=ig[:], op=ALU.add
                    )
            tc_t = tmp.tile([128, NJ, BC], F16, tag="tc", name=f"tc{t}")
            nc.scalar.activation(out=tc_t[:], in_=cn[:], func=AF.Tanh)
            hn = hcpool.tile([128, NJ, BC], F16, tag="h", name=f"h{t}")
            for j in range(NJ):
                nc.vector.tensor_tensor(
                    out=hn[:, j, :], in0=gq[(3, j)][:], in1=tc_t[:, j, :],
                    op=ALU.mult
                )
            return hn, cn

        h_t = None
        c_t = None
        for t in range(L1):
            h_t, c_t = emit_step(t, h_t, c_t)

        # ---- final h transposes: h_bmb[cb] = [128 b, 384] f16 ----
        h_bmb = [
            singles.tile([128, H], F16, tag=f"hbm{cb}", name=f"hbm{cb}")
            for cb in range(NB)
        ]
        for j in range(NJ):
            for cb in range(NB):
                ps = psum_tr.tile([128, 128], F16, tag="trps", name=f"htp{j}_{cb}")
                nc.tensor.transpose(
                    out=ps[:],
                    in_=h_t[:, j, cb * 128 : (cb + 1) * 128],
                    identity=ident_h[:],
                )
                nc.vector.tensor_copy(
                    out=h_bmb[cb][:, j * 128 : (j + 1) * 128], in_=ps[:]
                )

        # ---- window dots: s_j[b,k] = h_seg_j[b] . e2[b,k] ----
        s_t = []
        for cb in range(NB):
            s = small.tile([128, NJ, L2], F32, tag=f"s{cb}", name=f"s{cb}")
            s_t.append(s)
            g2cb = g2[:, cb * L2 : (cb + 1) * L2, :]
            for j in range(NJ):
                prod = prodp.tile([128, L2, D], F16, tag="prod", name=f"pr{cb}_{j}")
                hseg = h_bmb[cb][:, j * 128 : (j + 1) * 128]
                hbc = bass.AP(
                    tensor=hseg.tensor,
                    offset=hseg.offset,
                    ap=[hseg.ap[0], [0, L2], hseg.ap[1]],
                )
                nc.vector.tensor_tensor(out=prod[:], in0=g2cb, in1=hbc, op=ALU.mult)
                # tree-sum: two f16 half-adds (2x mode), then a short reduce
                h1 = prodp.tile([128, L2, D // 2], F16, tag="half1",
                                name=f"h1_{cb}_{j}")
                nc.vector.tensor_tensor(
                    out=h1[:], in0=prod[:, :, 0 : D // 2],
                    in1=prod[:, :, D // 2 : D], op=ALU.add
                )
                h2 = prodp.tile([128, L2, D // 4], F16, tag="half2",
                                name=f"h2_{cb}_{j}")
                nc.vector.tensor_tensor(
                    out=h2[:], in0=h1[:, :, 0 : D // 4],
                    in1=h1[:, :, D // 4 : D // 2], op=ALU.add
                )
                nc.vector.tensor_reduce(
                    out=s[:, j, :], in_=h2[:], axis=AX.X, op=ALU.add
                )

        # ---- windows max + log-softmax (exp's batched, then ln's) ----
        a_t, negm_t, se_t, lse_t = [], [], [], []
        for cb in range(NB):
            s = s_t[cb]
            rs0 = small.tile([128, NWIN], F32, tag="rs0", name=f"rs0_{cb}")
            nc.vector.tensor_tensor(
                out=rs0[:], in0=s[:, 0, 0:NWIN], in1=s[:, 1, 1 : NWIN + 1], op=ALU.add
            )
            rs1 = small.tile([128, NWIN], F32, tag="rs1", name=f"rs1_{cb}")
            nc.vector.tensor_tensor(
                out=rs1[:], in0=rs0[:], in1=s[:, 2, 2 : NWIN + 2], op=ALU.add
            )
            ms = small.tile([128, 1], F32, tag="ms", name=f"ms{cb}")
            nc.vector.tensor_reduce(out=ms[:], in_=rs1[:], axis=AX.X, op=ALU.max)
            a = small.tile([128, OUT], F32, tag=f"a{cb}", name=f"a{cb}")
            nc.vector.scalar_tensor_tensor(
                out=a[:],
                in0=lwb_sb[:, 0:2],
                scalar=ms[:, 0:1],
                in1=lwb_sb[:, 2:4],
                op0=ALU.mult,
                op1=ALU.add,
            )
            negm = small.tile([128, 1], F32, tag=f"negm{cb}", name=f"negm{cb}")
            nc.vector.tensor_reduce(
                out=negm[:], in_=a[:], axis=AX.X, op=ALU.max, negate=True
            )
            a_t.append(a)
            negm_t.append(negm)
        for cb in range(NB):
            e = small.tile([128, OUT], F32, tag=f"e{cb}", name=f"e{cb}")
            se = small.tile([128, 1], F32, tag=f"se{cb}", name=f"se{cb}")
            nc.scalar.activation(
                out=e[:], in_=a_t[cb][:], func=AF.Exp, bias=negm_t[cb][:, 0:1],
                accum_out=se[:],
            )
            se_t.append(se)
        for cb in range(NB):
            lse = small.tile([128, 1], F32, tag=f"lse{cb}", name=f"lse{cb}")
            nc.scalar.activation(out=lse[:], in_=se_t[cb][:], func=AF.Ln)
            lse_t.append(lse)
        for cb in range(NB):
            combo = small.tile([128, 1], F32, tag=f"combo{cb}", name=f"combo{cb}")
            nc.vector.tensor_tensor(
                out=combo[:], in0=negm_t[cb][:], in1=lse_t[cb][:], op=ALU.subtract
            )
            ot = small.tile([128, OUT], F32, tag=f"ot{cb}", name=f"ot{cb}")
            nc.vector.tensor_scalar_add(ot[:], a_t[cb][:], combo[:, 0:1])
            nc.sync.dma_start(out=out_d[cb * 128 : (cb + 1) * 128, :], in_=ot[:])

    nc.compile()
    _cache["nc"] = nc
    return nc


def _wrap16(flat):
    """idx i -> partition i%16, col i//16; replicated to 128 partitions."""
    n = len(flat)
    a = np.zeros((16, n // 16), np.int16)
    a[np.arange(n) % 16, np.arange(n) // 16] = flat.astype(np.int16)
    return np.tile(a, (8, 1))


def kernel(input1, input2, emb, W_ih, W_hh, b_ih, b_hh, lin_w, lin_b, _trace=False):
    input1 = np.ascontiguousarray(np.asarray(input1, dtype=np.int64))
    input2 = np.ascontiguousarray(np.asarray(input2, dtype=np.int64))
    emb = np.asarray(emb, dtype=np.float32)
    W_ih = np.asarray(W_ih, dtype=np.float32)
    W_hh = np.asarray(W_hh, dtype=np.float32)
    b = (np.asarray(b_ih, dtype=np.float32) + np.asarray(b_hh, dtype=np.float32))
    lin_w = np.asarray(lin_w, dtype=np.float32)
    lin_b = np.asarray(lin_b, dtype=np.float32)

    # weight layout: Wfull = [W_ih | W_hh] : [4H, 128+384]; lhsT tiles are
    # Wfull.T reshaped [4 k-chunks, 128, 4H]
    wfull = np.concatenate([W_ih, W_hh], axis=1)          # [1536, 512]
    wt = np.ascontiguousarray(wfull.T.reshape(4, 128, 4 * H).astype(np.float16))
    biasc = np.ascontiguousarray(b.reshape(4 * H // 128, 128))
    lwb = np.ascontiguousarray(
        np.array([[lin_w[0, 0], lin_w[1, 0], lin_b[0], lin_b[1]]], dtype=np.float32)
    )
    ident = np.eye(128, dtype=np.float16)

    nc = _build()

    in_maps = []
    for c in range(NCORES):
        i1 = input1[c * BC : (c + 1) * BC]                # [512, 19]
        i2 = input2[c * BC : (c + 1) * BC]                # [512, 20]
        ids = np.unique(np.concatenate([i1.ravel(), i2.ravel()]))
        assert len(ids) <= VC, len(ids)
        embc = np.zeros((VC, D), np.float16)
        embc[: len(ids)] = emb[ids].astype(np.float16)
        c1 = np.searchsorted(ids, i1)                     # [512, 19]
        c2 = np.searchsorted(ids, i2)                     # [512, 20]
        # e1 flat order per wave: i = b (= cb*128+p)
        ix1 = np.concatenate(
            [_wrap16(c1[:, w]) for w in range(L1)], axis=1
        )
        # e2 flat order per cb: i = p + 128*k
        ix2 = np.concatenate(
            [_wrap16(c2[cb * 128 : (cb + 1) * 128].T.ravel()) for cb in range(NB)],
            axis=1,
        )
        in_maps.append(
            {
                "emb": embc,
                "wt": wt,
                "biasc": biasc,
                "lwb": lwb,
                "ident": ident,
                "ix1": np.ascontiguousarray(ix1),
                "ix2": np.ascontiguousarray(ix2),
            }
        )

    res = bass_utils.run_bass_kernel_spmd(
        nc, in_maps, core_ids=list(range(NCORES)), trace=_trace
    )
    if _trace:
        kernel.last_results = res
    out = np.concatenate([res.results[c]["out"] for c in range(NCORES)], axis=0)
    return out


if __name__ == "__main__":
    rng = np.random.default_rng(0)
    inputs = {
        "input1": rng.integers(0, V, (B, L1), dtype=np.int32),
        "input2": rng.integers(0, V, (B, L2), dtype=np.int32),
        "emb": rng.standard_normal((V, D), dtype=np.float32),
        "W_ih": (rng.standard_normal((4 * H, D), dtype=np.float32) * 0.05),
        "W_hh": (rng.standard_normal((4 * H, H), dtype=np.float32) * 0.05),
        "b_ih": (rng.standard_normal(4 * H).astype(np.float32) * 0.05),
        "b_hh": (rng.standard_normal(4 * H).astype(np.float32) * 0.05),
        "lin_w": rng.standard_normal((OUT, 1), dtype=np.float32),
        "lin_b": rng.standard_normal(OUT).astype(np.float32),
    }
    out = kernel(**inputs)
    print(out.shape, out[:2])
